# revision 32
# baseline (speedup 1.0000x reference)
"""Causal self-attention Trainium2 kernel (B=1, S=4096, E=1024, H=16, D=64).

Sharding: tensor-parallel over heads — 2 heads per core (8 cores).
Each core computes Q/K/V for its 2 heads, causal attention, and a partial
o_proj over its 128 output-feature slice; the host sums the 8 partials.

Device-side structure (per core):
  * x arrives pre-transposed as xT [E, S] bf16 (host does the transpose),
    so every matmul contracts over the partition axis with contiguous DMAs.
  * Q/K kept transposed in SBUF (qts/kts: [128(d of 2 heads), 512] tiles);
    V in normal layout ([128(s), 64+1] tiles, ones column appended so the
    PV matmul also accumulates the softmax denominator in PSUM row 64).
  * Logits computed transposed, lg[kv, q] = K @ Q.T, both heads packed
    into PE row-groups (tile_position rows 0/64) writing separate banks.
  * exp on ScalarE over [128, 1024] PSUM->SBUF (scale folded in); no
    max-subtraction (logits ~ N(0,1)). Causal masking multiplies the 4
    diagonal-band blocks per q-tile by 0/1 masks.
  * Normalize via reciprocal_approx_fast + stream_shuffle broadcast.
  * The per-q-tile QKV projections and o_proj matmuls are interleaved as
    PE "filler" work between attention pairs so the PE never idles long
    enough for the HAM clock gate to re-throttle it.
"""

import math
import sys
from collections import deque

import numpy as np

for _p in ("/opt/trn_rl_repo", "/opt/trn_rl_repo/concourse"):
    if _p not in sys.path:
        sys.path.insert(0, _p)

import ml_dtypes

BF16 = ml_dtypes.bfloat16

S = 4096
E = 1024
H = 16
D = 64
NCORES = 8
DH = 128  # head dims per core (2 heads x 64)
QT = 512  # query tile (free dim of logits matmuls)
NQ = S // QT  # 8
KB = 128  # kv block (partition dim of logits tiles)
SCALE = 1.0 / math.sqrt(D)

_CACHE = {}


def _build_nc():
    import concourse.tile as tile
    from concourse import bacc, mybir

    dt = mybir.dt
    f32 = dt.float32
    bf16 = dt.bfloat16
    fp8 = dt.float8e4
    Exp = mybir.ActivationFunctionType.Exp
    DoubleRow = mybir.MatmulPerfMode.DoubleRow
    # exp(scale*logit + EXP_BIAS): keeps exp values well inside fp8e4's
    # +-240 range; the uniform e^bias factor cancels in the normalization.
    EXP_BIAS = -3.5

    nc = bacc.Bacc("TRN2", target_bir_lowering=False, debug=False, num_devices=NCORES)

    xT_d = nc.dram_tensor("xT", [E, S], bf16, kind="ExternalInput")
    wq_d = nc.dram_tensor("wq", [128, 1024], bf16, kind="ExternalInput")
    wk_d = nc.dram_tensor("wk", [128, 1024], bf16, kind="ExternalInput")
    wv_d = nc.dram_tensor("wv", [128, 1024], bf16, kind="ExternalInput")
    wo_d = nc.dram_tensor("wo", [128, 1024], bf16, kind="ExternalInput")
    out_d = nc.dram_tensor("out", [S, E], bf16, kind="ExternalOutput")

    with tile.TileContext(nc) as tc:
        from contextlib import ExitStack

        with ExitStack() as ctx:
            sb = ctx.enter_context(tc.tile_pool(name="sb", bufs=1))
            lgp = ctx.enter_context(tc.tile_pool(name="lgp", bufs=2, space="PSUM"))
            ps = ctx.enter_context(tc.tile_pool(name="ps", bufs=2, space="PSUM"))
            pvp = ctx.enter_context(tc.tile_pool(name="pvp", bufs=2, space="PSUM"))
            expp = ctx.enter_context(tc.tile_pool(name="expp", bufs=4))
            normp = ctx.enter_context(tc.tile_pool(name="normp", bufs=2))
            ostp = ctx.enter_context(tc.tile_pool(name="ostp", bufs=3))

            # ---- persistent SBUF tensors + input DMA ----
            # weights first (small, needed by the very first matmul), then the
            # first 512 columns of every xT chunk (enough for the q-tile-0
            # projections), then the bulk of xT.
            wq_sb = sb.tile([128, 1024], bf16, name="wq_sb", tag="wq_sb")
            wk_sb = sb.tile([128, 1024], bf16, name="wk_sb", tag="wk_sb")
            wv_sb = sb.tile([128, 1024], bf16, name="wv_sb", tag="wv_sb")
            wo_sb = sb.tile([128, 1024], bf16, name="wo_sb", tag="wo_sb")
            nc.sync.dma_start(wk_sb[:], wk_d[:])
            nc.sync.dma_start(wq_sb[:], wq_d[:])
            nc.sync.dma_start(wv_sb[:], wv_d[:])
            nc.sync.dma_start(wo_sb[:], wo_d[:])

            xts = [
                sb.tile([128, S], bf16, name=f"xt{ec}", tag=f"xt{ec}")
                for ec in range(8)
            ]
            for ec in range(8):
                nc.sync.dma_start(
                    xts[ec][:, 0:QT], xT_d[ec * 128 : (ec + 1) * 128, 0:QT]
                )
            for ec in range(8):
                nc.sync.dma_start(
                    xts[ec][:, QT:S], xT_d[ec * 128 : (ec + 1) * 128, QT:S]
                )

            kts = [sb.tile([128, QT], bf16, name=f"kt{i}", tag=f"kt{i}") for i in range(NQ)]
            qts = [sb.tile([128, QT], bf16, name=f"qt{i}", tag=f"qt{i}") for i in range(NQ)]
            aots = [sb.tile([128, QT], bf16, name=f"ao{i}", tag=f"ao{i}") for i in range(NQ)]
            # V for DoubleRow PV: one fp8 tile per kv-block PAIR, layout
            # [128(s within block), pair-slot(2) x 160]: head A V at d 0-63 +
            # ones col 64; head B V at 80-143 + ones col 144 (pair-slot
            # stride 160 B keeps the DoubleRow 16B-alignment rule).
            v8s = []
            for i in range(16):
                v = sb.tile([128, 320], fp8, name=f"v{i}", tag=f"v{i}")
                vv = v[:].rearrange("p (t d) -> p t d", t=2)
                nc.vector.memset(vv[:, :, 64:65], 1.0)
                nc.vector.memset(vv[:, :, 144:145], 1.0)
                v8s.append(v)
            # bf16 V for q-tile 0 (its rows have little context, so fp8
            # attention noise doesn't average out there -> keep bf16)
            vb16 = []
            for i in range(4):
                v = sb.tile([128, 130], bf16, name=f"vb{i}", tag=f"vb{i}")
                nc.vector.memset(v[:, 64:65], 1.0)
                nc.vector.memset(v[:, 129:130], 1.0)
                vb16.append(v)

            # seed tile for the denominator-reciprocal broadcast
            bcseed = sb.tile([64, QT], f32, name="bcseed", tag="bcseed")
            nc.vector.memset(bcseed[:], 0.0)
            # per-partition bias column for the exp range shift
            ebias = sb.tile([128, 1], f32, name="ebias", tag="ebias")
            nc.vector.memset(ebias[:], EXP_BIAS)

            # ---- filler-unit constructors (projections / o_proj) ----
            def kq_units(dst, w, st):
                cols = slice(st * QT, (st + 1) * QT)
                state = {}

                def mm(ec):
                    def f():
                        if ec == 0:
                            state["t"] = ps.tile([128, QT], f32, name="ps_kq", tag="ps")
                        nc.tensor.matmul(
                            state["t"][:],
                            lhsT=w[:, ec * 128 : (ec + 1) * 128],
                            rhs=xts[ec][:, cols],
                            start=(ec == 0),
                            stop=(ec == 7),
                        )

                    return f

                def cast():
                    nc.vector.tensor_copy(dst[:], state["t"][:])

                return [mm(ec) for ec in range(8)] + [cast]

            def v_units(kb):
                state = {}

                def mm(ec):
                    def f():
                        if ec == 0:
                            state["t"] = ps.tile([128, 128], f32, name="ps_v", tag="ps")
                        nc.tensor.matmul(
                            state["t"][:],
                            lhsT=xts[ec][:, kb * 128 : (kb + 1) * 128],
                            rhs=wv_sb[:, ec * 128 : (ec + 1) * 128],
                            start=(ec == 0),
                            stop=(ec == 7),
                        )

                    return f

                def cast():
                    vv = v8s[kb // 2][:].rearrange("p (t d) -> p t d", t=2)
                    r = kb % 2
                    nc.vector.tensor_copy(vv[:, r, 0:64], state["t"][:, 0:64])
                    nc.vector.tensor_copy(vv[:, r, 80:144], state["t"][:, 64:128])
                    if kb < 4:
                        nc.vector.tensor_copy(vb16[kb][:, 0:64], state["t"][:, 0:64])
                        nc.vector.tensor_copy(vb16[kb][:, 65:129], state["t"][:, 64:128])

                return [mm(ec) for ec in range(8)] + [cast]

            def oproj_units(qj):
                units = []
                for sbi in range(4):
                    for half in range(2):

                        def f(sbi=sbi, half=half):
                            srow = qj * QT + sbi * 128
                            po = ps.tile([128, 512], f32, name="po", tag="ps")
                            nc.tensor.matmul(
                                po[:],
                                lhsT=aots[qj][:, sbi * 128 : (sbi + 1) * 128],
                                rhs=wo_sb[:, half * 512 : (half + 1) * 512],
                                start=True,
                                stop=True,
                            )
                            ost = ostp.tile([128, 512], bf16, name="ost", tag="ost")
                            nc.vector.tensor_copy(ost[:], po[:])
                            nc.sync.dma_start(
                                out_d[srow : srow + 128, half * 512 : (half + 1) * 512],
                                ost[:],
                            )

                        units.append(f)
                return units

            def proj_units(qi2):
                u = []
                u += kq_units(kts[qi2], wk_sb, qi2)
                u += kq_units(qts[qi2], wq_sb, qi2)
                for kb in range(4 * qi2, 4 * qi2 + 4):
                    u += v_units(kb)
                return u

            # ---- prologue ----
            # q-tiles are processed in order [1, 2, ..., 7, 0]: starting with
            # tile 1 gives ACT twice the early exp work, and finishing with
            # tile 0 (only 4 kv blocks) makes the serial tail short. The
            # prologue projects just what tile 1's first blocks need; V
            # blocks 2..7 stream in as fillers during tile 1 itself.
            ORDER = [1, 2, 3, 4, 5, 6, 7, 0]
            for f in (
                kq_units(kts[0], wk_sb, 0)
                + kq_units(kts[1], wk_sb, 1)
                + kq_units(qts[1], wq_sb, 1)
                + v_units(0)
                + v_units(1)
            ):
                f()

            # ---- main loop over q-tiles ----
            for idx, qi in enumerate(ORDER):
                fillers = deque()
                if 1 <= qi <= 6:
                    # projections for the next processed tile (qi+1); tile
                    # 1's own V blocks 2..7 are emitted inline in its loop
                    fillers.extend(kq_units(kts[qi + 1], wk_sb, qi + 1))
                    fillers.extend(kq_units(qts[qi + 1], wq_sb, qi + 1))
                    for kb in range(4 * qi + 4, 4 * qi + 8):
                        fillers.extend(v_units(kb))
                elif qi == 7:
                    fillers.extend(kq_units(qts[0], wq_sb, 0))
                if idx >= 1:
                    fillers.extend(oproj_units(ORDER[idx - 1]))

                n_kb = 4 * (qi + 1)
                pvA = pvp.tile([65, QT], f32, name="pvA", tag="pv")
                pvB = pvp.tile([65, QT], f32, name="pvB", tag="pv")
                if qi == 0:
                    # bf16 path for the first q-tile (rows 0-511): every
                    # block is diagonal; per-block M=65 PV, no DoubleRow.
                    for kb in range(4):
                        kvs = slice(kb * KB, (kb + 1) * KB)
                        off = kb
                        qlo = off * KB
                        nq = QT - qlo
                        lg = lgp.tile([128, 2 * QT], f32, name="lg", tag="lg")
                        nc.tensor.matmul(
                            lg[:, qlo:QT], lhsT=kts[0][0:64, kvs],
                            rhs=qts[0][0:64, qlo:QT], start=True, stop=True,
                        )
                        nc.tensor.matmul(
                            lg[:, QT + qlo : 2 * QT], lhsT=kts[0][64:128, kvs],
                            rhs=qts[0][64:128, qlo:QT], start=True, stop=True,
                        )
                        exb = expp.tile([128, 2 * QT], bf16, name="exb", tag="exp")
                        lg_v = lg[:].rearrange("p (h q) -> p h q", h=2)[:, :, qlo:QT]
                        exb_v = exb[:].rearrange("p (h q) -> p h q", h=2)[:, :, qlo:QT]
                        nc.scalar.activation(exb_v, lg_v, Exp, scale=SCALE,
                                             bias=ebias[:])
                        if fillers:
                            n_pop = math.ceil(len(fillers) / (4 - kb))
                            for _ in range(n_pop):
                                fillers.popleft()()
                        nc.gpsimd.affine_select(
                            out=exb_v, in_=exb_v,
                            compare_op=mybir.AluOpType.is_ge,
                            fill=0.0, base=0,
                            pattern=[[0, 2], [1, nq]],
                            channel_multiplier=-1,
                        )
                        nc.tensor.matmul(
                            pvA[:, qlo:QT], lhsT=vb16[kb][:, 0:65],
                            rhs=exb[:, qlo:QT],
                            start=(kb == 0), stop=(kb == 3),
                            skip_group_check=True,
                        )
                        nc.tensor.matmul(
                            pvB[:, qlo:QT], lhsT=vb16[kb][:, 65:130],
                            rhs=exb[:, QT + qlo : 2 * QT],
                            start=(kb == 0), stop=(kb == 3),
                            skip_group_check=True,
                        )
                    n_kb = 0  # skip the fp8 loop below

                ex8 = None
                for kb in range(n_kb):
                    # logits for both heads of kv-block kb: head A -> cols
                    # 0:512 (PSUM bank 0), head B -> cols 512:1024 (bank 1).
                    # Row-group packing (rows 0-63 / 64-127) runs the two
                    # matmuls concurrently in the PE array.
                    # Diagonal-band pairs: columns q < qlo_p = (pair off)*128
                    # are entirely masked for both members -> skip them.
                    kvs = slice((kb % 4) * KB, (kb % 4 + 1) * KB)
                    ktile = kts[kb // 4]
                    r = kb % 2
                    off = kb - 4 * qi
                    qlo = max(off - r, 0) * KB  # pair-aligned trim
                    nq = QT - qlo
                    lg = lgp.tile([128, 2 * QT], f32, name="lg", tag="lg")
                    nc.tensor.matmul(
                        lg[:, qlo:QT], lhsT=ktile[0:64, kvs],
                        rhs=qts[qi][0:64, qlo:QT],
                        start=True, stop=True,
                    )
                    nc.tensor.matmul(
                        lg[:, QT + qlo : 2 * QT], lhsT=ktile[64:128, kvs],
                        rhs=qts[qi][64:128, qlo:QT],
                        start=True, stop=True,
                    )
                    if r == 0:
                        # exp tile for this kv pair: [128, (head, slot, q)]
                        ex8 = expp.tile([128, 4 * QT], fp8, name="ex8", tag="exp")
                    exv = ex8[:].rearrange("p (h t q) -> p h t q", h=2, t=2)
                    lg_v = lg[:].rearrange("p (h q) -> p h q", h=2)[:, :, qlo:QT]
                    ex_v = exv[:, :, r, qlo:QT]
                    nc.scalar.activation(ex_v, lg_v, Exp, scale=SCALE, bias=ebias[:])

                    # tile 1 streams its own remaining V blocks just ahead
                    # of the PV matmuls that consume them
                    if qi == 1 and kb + 2 < n_kb:
                        for f in v_units(kb + 2):
                            f()
                    # PE filler work while ACT computes exp
                    if fillers:
                        n_pop = math.ceil(len(fillers) / (n_kb - kb))
                        for _ in range(n_pop):
                            fillers.popleft()()

                    if off >= 0:
                        # causal mask on GPSIMD: keep where q - kv - off*128
                        # >= 0 else 0; with q = qlo + j this is
                        # j + qlo - off*128 - kv >= 0.
                        nc.gpsimd.affine_select(
                            out=ex_v,
                            in_=ex_v,
                            compare_op=mybir.AluOpType.is_ge,
                            fill=0.0,
                            base=qlo - off * KB,
                            pattern=[[0, 2], [1, nq]],
                            channel_multiplier=-1,
                        )
                    if r == 1:
                        # DoubleRow PV over the kv pair (contraction 256):
                        # lhsT [128, 2, 65], rhs [128, 2, nq] -> out [65, nq]
                        kp = kb // 2
                        vv = v8s[kp][:].rearrange("p (t d) -> p t d", t=2)
                        nc.tensor.matmul(
                            pvA[:, qlo:QT], lhsT=vv[:, :, 0:65],
                            rhs=exv[:, 0, :, qlo:QT],
                            start=(kp == 0), stop=(kb == n_kb - 1),
                            perf_mode=DoubleRow,
                            skip_group_check=True,
                        )
                        nc.tensor.matmul(
                            pvB[:, qlo:QT], lhsT=vv[:, :, 80:145],
                            rhs=exv[:, 1, :, qlo:QT],
                            start=(kp == 0), stop=(kb == n_kb - 1),
                            perf_mode=DoubleRow,
                            skip_group_check=True,
                        )
                while fillers:
                    fillers.popleft()()
                # normalize: aot = pv[0:64] / pv[64]
                for pv, r0 in ((pvA, 0), (pvB, 64)):
                    den_sb = normp.tile([1, QT], f32, name="den_sb", tag="den")
                    nc.vector.tensor_copy(den_sb[:], pv[64:65, :])
                    nc.vector.reciprocal_approx_fast(bcseed[0:1, :], den_sb[:])
                    nc.vector.tensor_copy(bcseed[32:33, :], bcseed[0:1, :])
                    bcast = normp.tile([64, QT], f32, name="bcast", tag="bcast")
                    nc.vector.stream_shuffle(bcast[:], bcseed[:], [0] * 32)
                    nc.vector.tensor_mul(aots[qi][r0 : r0 + 64, :], pv[0:64, :], bcast[:])

            # epilogue: o_proj of the last-processed q-tile (tile 0)
            for f in oproj_units(ORDER[-1]):
                f()

    nc.compile()
    return nc


def _host_inputs(x, Wq, Wk, Wv, Wo):
    x2 = np.asarray(x, dtype=np.float32).reshape(S, E)
    xT = np.ascontiguousarray(x2.T).astype(BF16)

    in_maps = []
    for c in range(NCORES):
        r = slice(128 * c, 128 * (c + 1))

        def pack(wT):  # [1024(e), 128(d)] -> [128(p), ec*128+d]
            return np.ascontiguousarray(
                wT.reshape(8, 128, 128).transpose(1, 0, 2).reshape(128, 1024)
            ).astype(BF16)

        wq_c = pack(np.asarray(Wq, np.float32)[r, :].T)
        wk_c = pack(np.asarray(Wk, np.float32)[r, :].T)
        wv_c = pack(np.asarray(Wv, np.float32)[r, :].T)
        wo_c = np.ascontiguousarray(np.asarray(Wo, np.float32)[:, r].T).astype(BF16)
        in_maps.append(
            {
                "xT": xT,
                "wq": wq_c,
                "wk": wk_c,
                "wv": wv_c,
                "wo": wo_c,
            }
        )
    return in_maps


def _get_nc():
    if "nc" not in _CACHE:
        _CACHE["nc"] = _build_nc()
    return _CACHE["nc"]


def run(x, Wq, Wk, Wv, Wo, trace=False, trace_kwargs=None):
    """Build+run the SPMD kernel; returns (full_output [S,E] f32, BassKernelResults)."""
    from concourse.bass_utils import run_bass_kernel_spmd

    nc = _get_nc()
    in_maps = _host_inputs(x, Wq, Wk, Wv, Wo)
    res = run_bass_kernel_spmd(
        nc,
        in_maps,
        list(range(NCORES)),
        trace=trace,
        **(trace_kwargs or {}),
    )
    out = np.zeros((S, E), dtype=np.float32)
    for c in range(NCORES):
        out += res.results[c]["out"].astype(np.float32)
    return out, res


def kernel(x, Wq, Wk, Wv, Wo):
    out, _ = run(x, Wq, Wk, Wv, Wo)
    return out.reshape(1, S, E).astype(np.float32)


# revision 33
# speedup vs baseline: 1.0206x; 1.0206x over previous
"""Causal self-attention Trainium2 kernel (B=1, S=4096, E=1024, H=16, D=64).

Sharding: tensor-parallel over heads — 2 heads per core (8 cores).
Each core computes Q/K/V for its 2 heads, causal attention, and a partial
o_proj over its 128 output-feature slice; the host sums the 8 partials.

Device-side structure (per core):
  * x arrives pre-transposed as xT [E, S] bf16 (host does the transpose),
    so every matmul contracts over the partition axis with contiguous DMAs.
  * Q/K kept transposed in SBUF (qts/kts: [128(d of 2 heads), 512] tiles);
    V in normal layout ([128(s), 64+1] tiles, ones column appended so the
    PV matmul also accumulates the softmax denominator in PSUM row 64).
  * Logits computed transposed, lg[kv, q] = K @ Q.T, both heads packed
    into PE row-groups (tile_position rows 0/64) writing separate banks.
  * exp on ScalarE over [128, 1024] PSUM->SBUF (scale folded in); no
    max-subtraction (logits ~ N(0,1)). Causal masking multiplies the 4
    diagonal-band blocks per q-tile by 0/1 masks.
  * Normalize via reciprocal_approx_fast + stream_shuffle broadcast.
  * The per-q-tile QKV projections and o_proj matmuls are interleaved as
    PE "filler" work between attention pairs so the PE never idles long
    enough for the HAM clock gate to re-throttle it.
"""

import math
import sys
from collections import deque

import numpy as np

for _p in ("/opt/trn_rl_repo", "/opt/trn_rl_repo/concourse"):
    if _p not in sys.path:
        sys.path.insert(0, _p)

import ml_dtypes

BF16 = ml_dtypes.bfloat16

S = 4096
E = 1024
H = 16
D = 64
NCORES = 8
DH = 128  # head dims per core (2 heads x 64)
QT = 512  # query tile (free dim of logits matmuls)
NQ = S // QT  # 8
KB = 128  # kv block (partition dim of logits tiles)
SCALE = 1.0 / math.sqrt(D)

_CACHE = {}


def _build_nc():
    import concourse.tile as tile
    from concourse import bacc, mybir

    dt = mybir.dt
    f32 = dt.float32
    bf16 = dt.bfloat16
    fp8 = dt.float8e4
    Exp = mybir.ActivationFunctionType.Exp
    DoubleRow = mybir.MatmulPerfMode.DoubleRow
    # exp(scale*logit + EXP_BIAS): keeps exp values well inside fp8e4's
    # +-240 range; the uniform e^bias factor cancels in the normalization.
    EXP_BIAS = -3.5

    nc = bacc.Bacc("TRN2", target_bir_lowering=False, debug=False, num_devices=NCORES)

    xT_d = nc.dram_tensor("xT", [E, S], bf16, kind="ExternalInput")
    wq_d = nc.dram_tensor("wq", [128, 1024], bf16, kind="ExternalInput")
    wk_d = nc.dram_tensor("wk", [128, 1024], bf16, kind="ExternalInput")
    wv_d = nc.dram_tensor("wv", [128, 1024], bf16, kind="ExternalInput")
    wo_d = nc.dram_tensor("wo", [128, 1024], bf16, kind="ExternalInput")
    out_d = nc.dram_tensor("out", [S, E], bf16, kind="ExternalOutput")

    with tile.TileContext(nc) as tc:
        from contextlib import ExitStack

        with ExitStack() as ctx:
            sb = ctx.enter_context(tc.tile_pool(name="sb", bufs=1))
            lgp = ctx.enter_context(tc.tile_pool(name="lgp", bufs=2, space="PSUM"))
            ps = ctx.enter_context(tc.tile_pool(name="ps", bufs=2, space="PSUM"))
            pvp = ctx.enter_context(tc.tile_pool(name="pvp", bufs=2, space="PSUM"))
            expp = ctx.enter_context(tc.tile_pool(name="expp", bufs=4))
            normp = ctx.enter_context(tc.tile_pool(name="normp", bufs=2))
            ostp = ctx.enter_context(tc.tile_pool(name="ostp", bufs=3))

            # ---- persistent SBUF tensors + input DMA ----
            # weights first (small, needed by the very first matmul), then the
            # first 512 columns of every xT chunk (enough for the q-tile-0
            # projections), then the bulk of xT.
            wq_sb = sb.tile([128, 1024], bf16, name="wq_sb", tag="wq_sb")
            wk_sb = sb.tile([128, 1024], bf16, name="wk_sb", tag="wk_sb")
            wv_sb = sb.tile([128, 1024], bf16, name="wv_sb", tag="wv_sb")
            wo_sb = sb.tile([128, 1024], bf16, name="wo_sb", tag="wo_sb")
            nc.sync.dma_start(wk_sb[:], wk_d[:])
            nc.sync.dma_start(wq_sb[:], wq_d[:])
            nc.sync.dma_start(wv_sb[:], wv_d[:])
            nc.sync.dma_start(wo_sb[:], wo_d[:])

            xts = [
                sb.tile([128, S], bf16, name=f"xt{ec}", tag=f"xt{ec}")
                for ec in range(8)
            ]
            for ec in range(8):
                nc.sync.dma_start(
                    xts[ec][:, 0 : 2 * QT], xT_d[ec * 128 : (ec + 1) * 128, 0 : 2 * QT]
                )
            for ec in range(8):
                nc.sync.dma_start(
                    xts[ec][:, 2 * QT : S], xT_d[ec * 128 : (ec + 1) * 128, 2 * QT : S]
                )

            kts = [sb.tile([128, QT], bf16, name=f"kt{i}", tag=f"kt{i}") for i in range(NQ)]
            qts = [sb.tile([128, QT], bf16, name=f"qt{i}", tag=f"qt{i}") for i in range(NQ)]
            aots = [sb.tile([128, QT], bf16, name=f"ao{i}", tag=f"ao{i}") for i in range(NQ)]
            # V for DoubleRow PV: one fp8 tile per kv-block PAIR, layout
            # [128(s within block), pair-slot(2) x 160]: head A V at d 0-63 +
            # ones col 64; head B V at 80-143 + ones col 144 (pair-slot
            # stride 160 B keeps the DoubleRow 16B-alignment rule).
            v8s = []
            for i in range(16):
                v = sb.tile([128, 320], fp8, name=f"v{i}", tag=f"v{i}")
                vv = v[:].rearrange("p (t d) -> p t d", t=2)
                nc.vector.memset(vv[:, :, 64:65], 1.0)
                nc.vector.memset(vv[:, :, 144:145], 1.0)
                v8s.append(v)
            # bf16 V for q-tile 0 (its rows have little context, so fp8
            # attention noise doesn't average out there -> keep bf16)
            vb16 = []
            for i in range(4):
                v = sb.tile([128, 130], bf16, name=f"vb{i}", tag=f"vb{i}")
                nc.vector.memset(v[:, 64:65], 1.0)
                nc.vector.memset(v[:, 129:130], 1.0)
                vb16.append(v)

            # seed tile for the denominator-reciprocal broadcast
            bcseed = sb.tile([64, QT], f32, name="bcseed", tag="bcseed")
            nc.vector.memset(bcseed[:], 0.0)
            # per-partition bias column for the exp range shift
            ebias = sb.tile([128, 1], f32, name="ebias", tag="ebias")
            nc.vector.memset(ebias[:], EXP_BIAS)

            # ---- filler-unit constructors (projections / o_proj) ----
            def kq_units(dst, w, st):
                cols = slice(st * QT, (st + 1) * QT)
                state = {}

                def mm(ec):
                    def f():
                        if ec == 0:
                            state["t"] = ps.tile([128, QT], f32, name="ps_kq", tag="ps")
                        nc.tensor.matmul(
                            state["t"][:],
                            lhsT=w[:, ec * 128 : (ec + 1) * 128],
                            rhs=xts[ec][:, cols],
                            start=(ec == 0),
                            stop=(ec == 7),
                        )

                    return f

                def cast():
                    nc.vector.tensor_copy(dst[:], state["t"][:])

                return [mm(ec) for ec in range(8)] + [cast]

            def v_units(kb):
                state = {}

                def mm(ec):
                    def f():
                        if ec == 0:
                            state["t"] = ps.tile([128, 128], f32, name="ps_v", tag="ps")
                        nc.tensor.matmul(
                            state["t"][:],
                            lhsT=xts[ec][:, kb * 128 : (kb + 1) * 128],
                            rhs=wv_sb[:, ec * 128 : (ec + 1) * 128],
                            start=(ec == 0),
                            stop=(ec == 7),
                        )

                    return f

                def cast():
                    vv = v8s[kb // 2][:].rearrange("p (t d) -> p t d", t=2)
                    r = kb % 2
                    nc.vector.tensor_copy(vv[:, r, 0:64], state["t"][:, 0:64])
                    nc.vector.tensor_copy(vv[:, r, 80:144], state["t"][:, 64:128])
                    if kb < 4:
                        nc.vector.tensor_copy(vb16[kb][:, 0:64], state["t"][:, 0:64])
                        nc.vector.tensor_copy(vb16[kb][:, 65:129], state["t"][:, 64:128])

                return [mm(ec) for ec in range(8)] + [cast]

            def oproj_units(qj):
                units = []
                for sbi in range(4):
                    for half in range(2):

                        def f(sbi=sbi, half=half):
                            srow = qj * QT + sbi * 128
                            po = ps.tile([128, 512], f32, name="po", tag="ps")
                            nc.tensor.matmul(
                                po[:],
                                lhsT=aots[qj][:, sbi * 128 : (sbi + 1) * 128],
                                rhs=wo_sb[:, half * 512 : (half + 1) * 512],
                                start=True,
                                stop=True,
                            )
                            ost = ostp.tile([128, 512], bf16, name="ost", tag="ost")
                            nc.vector.tensor_copy(ost[:], po[:])
                            nc.sync.dma_start(
                                out_d[srow : srow + 128, half * 512 : (half + 1) * 512],
                                ost[:],
                            )

                        units.append(f)
                return units

            def proj_units(qi2):
                u = []
                u += kq_units(kts[qi2], wk_sb, qi2)
                u += kq_units(qts[qi2], wq_sb, qi2)
                for kb in range(4 * qi2, 4 * qi2 + 4):
                    u += v_units(kb)
                return u

            # ---- prologue ----
            # q-tiles are processed in order [1, 2, ..., 7, 0]: starting with
            # tile 1 gives ACT twice the early exp work, and finishing with
            # tile 0 (only 4 kv blocks) makes the serial tail short. The
            # prologue projects just what tile 1's first blocks need; V
            # blocks 2..7 stream in as fillers during tile 1 itself.
            ORDER = [1, 2, 3, 4, 5, 6, 7, 0]
            for f in (
                kq_units(kts[0], wk_sb, 0)
                + kq_units(kts[1], wk_sb, 1)
                + kq_units(qts[1], wq_sb, 1)
                + v_units(0)
                + v_units(1)
            ):
                f()

            # ---- main loop over q-tiles ----
            for idx, qi in enumerate(ORDER):
                fillers = deque()
                if 1 <= qi <= 6:
                    # projections for the next processed tile (qi+1); tile
                    # 1's own V blocks 2..7 are emitted inline in its loop
                    fillers.extend(kq_units(kts[qi + 1], wk_sb, qi + 1))
                    fillers.extend(kq_units(qts[qi + 1], wq_sb, qi + 1))
                    for kb in range(4 * qi + 4, 4 * qi + 8):
                        fillers.extend(v_units(kb))
                elif qi == 7:
                    fillers.extend(kq_units(qts[0], wq_sb, 0))
                if idx >= 1:
                    fillers.extend(oproj_units(ORDER[idx - 1]))

                n_kb = 4 * (qi + 1)
                pvA = pvp.tile([65, QT], f32, name="pvA", tag="pv")
                pvB = pvp.tile([65, QT], f32, name="pvB", tag="pv")
                if qi == 0:
                    # bf16 path for the first q-tile (rows 0-511): every
                    # block is diagonal; per-block M=65 PV, no DoubleRow.
                    for kb in range(4):
                        kvs = slice(kb * KB, (kb + 1) * KB)
                        off = kb
                        qlo = off * KB
                        nq = QT - qlo
                        lg = lgp.tile([128, 2 * QT], f32, name="lg", tag="lg")
                        nc.tensor.matmul(
                            lg[:, qlo:QT], lhsT=kts[0][0:64, kvs],
                            rhs=qts[0][0:64, qlo:QT], start=True, stop=True,
                        )
                        nc.tensor.matmul(
                            lg[:, QT + qlo : 2 * QT], lhsT=kts[0][64:128, kvs],
                            rhs=qts[0][64:128, qlo:QT], start=True, stop=True,
                        )
                        exb = expp.tile([128, 2 * QT], bf16, name="exb", tag="exp")
                        lg_v = lg[:].rearrange("p (h q) -> p h q", h=2)[:, :, qlo:QT]
                        exb_v = exb[:].rearrange("p (h q) -> p h q", h=2)[:, :, qlo:QT]
                        nc.scalar.activation(exb_v, lg_v, Exp, scale=SCALE,
                                             bias=ebias[:])
                        if fillers:
                            n_pop = math.ceil(len(fillers) / (4 - kb))
                            for _ in range(n_pop):
                                fillers.popleft()()
                        nc.gpsimd.affine_select(
                            out=exb_v, in_=exb_v,
                            compare_op=mybir.AluOpType.is_ge,
                            fill=0.0, base=0,
                            pattern=[[0, 2], [1, nq]],
                            channel_multiplier=-1,
                        )
                        nc.tensor.matmul(
                            pvA[:, qlo:QT], lhsT=vb16[kb][:, 0:65],
                            rhs=exb[:, qlo:QT],
                            start=(kb == 0), stop=(kb == 3),
                            skip_group_check=True,
                        )
                        nc.tensor.matmul(
                            pvB[:, qlo:QT], lhsT=vb16[kb][:, 65:130],
                            rhs=exb[:, QT + qlo : 2 * QT],
                            start=(kb == 0), stop=(kb == 3),
                            skip_group_check=True,
                        )
                    n_kb = 0  # skip the fp8 loop below

                ex8 = None
                for kb in range(n_kb):
                    # logits for both heads of kv-block kb: head A -> cols
                    # 0:512 (PSUM bank 0), head B -> cols 512:1024 (bank 1).
                    # Row-group packing (rows 0-63 / 64-127) runs the two
                    # matmuls concurrently in the PE array.
                    # Diagonal-band pairs: columns q < qlo_p = (pair off)*128
                    # are entirely masked for both members -> skip them.
                    kvs = slice((kb % 4) * KB, (kb % 4 + 1) * KB)
                    ktile = kts[kb // 4]
                    r = kb % 2
                    off = kb - 4 * qi
                    qlo = max(off - r, 0) * KB  # pair-aligned trim
                    nq = QT - qlo
                    lg = lgp.tile([128, 2 * QT], f32, name="lg", tag="lg")
                    nc.tensor.matmul(
                        lg[:, qlo:QT], lhsT=ktile[0:64, kvs],
                        rhs=qts[qi][0:64, qlo:QT],
                        start=True, stop=True,
                    )
                    nc.tensor.matmul(
                        lg[:, QT + qlo : 2 * QT], lhsT=ktile[64:128, kvs],
                        rhs=qts[qi][64:128, qlo:QT],
                        start=True, stop=True,
                    )
                    if r == 0:
                        # exp tile for this kv pair: [128, (head, slot, q)]
                        ex8 = expp.tile([128, 4 * QT], fp8, name="ex8", tag="exp")
                    exv = ex8[:].rearrange("p (h t q) -> p h t q", h=2, t=2)
                    lg_v = lg[:].rearrange("p (h q) -> p h q", h=2)[:, :, qlo:QT]
                    ex_v = exv[:, :, r, qlo:QT]
                    nc.scalar.activation(ex_v, lg_v, Exp, scale=SCALE, bias=ebias[:])

                    # tile 1 streams its own remaining V blocks just ahead
                    # of the PV matmuls that consume them
                    if qi == 1 and kb + 2 < n_kb:
                        for f in v_units(kb + 2):
                            f()
                    # PE filler work while ACT computes exp
                    if fillers:
                        n_pop = math.ceil(len(fillers) / (n_kb - kb))
                        for _ in range(n_pop):
                            fillers.popleft()()

                    if off >= 0:
                        # causal mask on GPSIMD: keep where q - kv - off*128
                        # >= 0 else 0; with q = qlo + j this is
                        # j + qlo - off*128 - kv >= 0.
                        nc.gpsimd.affine_select(
                            out=ex_v,
                            in_=ex_v,
                            compare_op=mybir.AluOpType.is_ge,
                            fill=0.0,
                            base=qlo - off * KB,
                            pattern=[[0, 2], [1, nq]],
                            channel_multiplier=-1,
                        )
                    if r == 1:
                        # DoubleRow PV over the kv pair (contraction 256):
                        # lhsT [128, 2, 65], rhs [128, 2, nq] -> out [65, nq]
                        kp = kb // 2
                        vv = v8s[kp][:].rearrange("p (t d) -> p t d", t=2)
                        nc.tensor.matmul(
                            pvA[:, qlo:QT], lhsT=vv[:, :, 0:65],
                            rhs=exv[:, 0, :, qlo:QT],
                            start=(kp == 0), stop=(kb == n_kb - 1),
                            perf_mode=DoubleRow,
                            skip_group_check=True,
                        )
                        nc.tensor.matmul(
                            pvB[:, qlo:QT], lhsT=vv[:, :, 80:145],
                            rhs=exv[:, 1, :, qlo:QT],
                            start=(kp == 0), stop=(kb == n_kb - 1),
                            perf_mode=DoubleRow,
                            skip_group_check=True,
                        )
                while fillers:
                    fillers.popleft()()
                # normalize: aot = pv[0:64] / pv[64]
                for pv, r0 in ((pvA, 0), (pvB, 64)):
                    den_sb = normp.tile([1, QT], f32, name="den_sb", tag="den")
                    nc.vector.tensor_copy(den_sb[:], pv[64:65, :])
                    nc.vector.reciprocal_approx_fast(bcseed[0:1, :], den_sb[:])
                    nc.vector.tensor_copy(bcseed[32:33, :], bcseed[0:1, :])
                    bcast = normp.tile([64, QT], f32, name="bcast", tag="bcast")
                    nc.vector.stream_shuffle(bcast[:], bcseed[:], [0] * 32)
                    nc.vector.tensor_mul(aots[qi][r0 : r0 + 64, :], pv[0:64, :], bcast[:])

            # epilogue: o_proj of the last-processed q-tile (tile 0)
            for f in oproj_units(ORDER[-1]):
                f()

    nc.compile()
    return nc


def _host_inputs(x, Wq, Wk, Wv, Wo):
    x2 = np.asarray(x, dtype=np.float32).reshape(S, E)
    xT = np.ascontiguousarray(x2.T).astype(BF16)

    in_maps = []
    for c in range(NCORES):
        r = slice(128 * c, 128 * (c + 1))

        def pack(wT):  # [1024(e), 128(d)] -> [128(p), ec*128+d]
            return np.ascontiguousarray(
                wT.reshape(8, 128, 128).transpose(1, 0, 2).reshape(128, 1024)
            ).astype(BF16)

        wq_c = pack(np.asarray(Wq, np.float32)[r, :].T)
        wk_c = pack(np.asarray(Wk, np.float32)[r, :].T)
        wv_c = pack(np.asarray(Wv, np.float32)[r, :].T)
        wo_c = np.ascontiguousarray(np.asarray(Wo, np.float32)[:, r].T).astype(BF16)
        in_maps.append(
            {
                "xT": xT,
                "wq": wq_c,
                "wk": wk_c,
                "wv": wv_c,
                "wo": wo_c,
            }
        )
    return in_maps


def _get_nc():
    if "nc" not in _CACHE:
        _CACHE["nc"] = _build_nc()
    return _CACHE["nc"]


def run(x, Wq, Wk, Wv, Wo, trace=False, trace_kwargs=None):
    """Build+run the SPMD kernel; returns (full_output [S,E] f32, BassKernelResults)."""
    from concourse.bass_utils import run_bass_kernel_spmd

    nc = _get_nc()
    in_maps = _host_inputs(x, Wq, Wk, Wv, Wo)
    res = run_bass_kernel_spmd(
        nc,
        in_maps,
        list(range(NCORES)),
        trace=trace,
        **(trace_kwargs or {}),
    )
    out = np.zeros((S, E), dtype=np.float32)
    for c in range(NCORES):
        out += res.results[c]["out"].astype(np.float32)
    return out, res


def kernel(x, Wq, Wk, Wv, Wo):
    out, _ = run(x, Wq, Wk, Wv, Wo)
    return out.reshape(1, S, E).astype(np.float32)


# revision 38
# speedup vs baseline: 1.0621x; 1.0406x over previous
"""Causal self-attention Trainium2 kernel (B=1, S=4096, E=1024, H=16, D=64).

Sharding: tensor-parallel over heads — 2 heads per core (8 cores).
Each core computes Q/K/V for its 2 heads, causal attention, and a partial
o_proj over its 128 output-feature slice; the host sums the 8 partials.

Device-side structure (per core):
  * x arrives pre-transposed as xT [E, S] bf16 (host does the transpose),
    so every matmul contracts over the partition axis with contiguous DMAs.
  * Q/K kept transposed in SBUF (qts/kts: [128(d of 2 heads), 512] tiles);
    V in normal layout ([128(s), 64+1] tiles, ones column appended so the
    PV matmul also accumulates the softmax denominator in PSUM row 64).
  * Logits computed transposed, lg[kv, q] = K @ Q.T, both heads packed
    into PE row-groups (tile_position rows 0/64) writing separate banks.
  * exp on ScalarE over [128, 1024] PSUM->SBUF (scale folded in); no
    max-subtraction (logits ~ N(0,1)). Causal masking multiplies the 4
    diagonal-band blocks per q-tile by 0/1 masks.
  * Normalize via reciprocal_approx_fast + stream_shuffle broadcast.
  * The per-q-tile QKV projections and o_proj matmuls are interleaved as
    PE "filler" work between attention pairs so the PE never idles long
    enough for the HAM clock gate to re-throttle it.
"""

import math
import sys
from collections import deque

import numpy as np

for _p in ("/opt/trn_rl_repo", "/opt/trn_rl_repo/concourse"):
    if _p not in sys.path:
        sys.path.insert(0, _p)

import ml_dtypes

BF16 = ml_dtypes.bfloat16

S = 4096
E = 1024
H = 16
D = 64
NCORES = 8
DH = 128  # head dims per core (2 heads x 64)
QT = 512  # query tile (free dim of logits matmuls)
NQ = S // QT  # 8
KB = 128  # kv block (partition dim of logits tiles)
SCALE = 1.0 / math.sqrt(D)

_CACHE = {}


def _build_nc():
    import concourse.tile as tile
    from concourse import bacc, mybir

    dt = mybir.dt
    f32 = dt.float32
    bf16 = dt.bfloat16
    fp8 = dt.float8e4
    Exp = mybir.ActivationFunctionType.Exp
    DoubleRow = mybir.MatmulPerfMode.DoubleRow
    # exp(scale*logit + EXP_BIAS): keeps exp values well inside fp8e4's
    # +-240 range; the uniform e^bias factor cancels in the normalization.
    EXP_BIAS = -3.5

    nc = bacc.Bacc("TRN2", target_bir_lowering=False, debug=False, num_devices=NCORES)

    xT_d = nc.dram_tensor("xT", [E, S], bf16, kind="ExternalInput")
    wq_d = nc.dram_tensor("wq", [128, 1024], bf16, kind="ExternalInput")
    wk_d = nc.dram_tensor("wk", [128, 1024], bf16, kind="ExternalInput")
    wv_d = nc.dram_tensor("wv", [128, 1024], bf16, kind="ExternalInput")
    wo_d = nc.dram_tensor("wo", [128, 1024], bf16, kind="ExternalInput")
    out_d = nc.dram_tensor("out", [S, E], bf16, kind="ExternalOutput")

    with tile.TileContext(nc) as tc:
        from contextlib import ExitStack

        with ExitStack() as ctx:
            sb = ctx.enter_context(tc.tile_pool(name="sb", bufs=1))
            lgp = ctx.enter_context(tc.tile_pool(name="lgp", bufs=2, space="PSUM"))
            ps = ctx.enter_context(tc.tile_pool(name="ps", bufs=2, space="PSUM"))
            pvp = ctx.enter_context(tc.tile_pool(name="pvp", bufs=2, space="PSUM"))
            expp = ctx.enter_context(tc.tile_pool(name="expp", bufs=4))
            normp = ctx.enter_context(tc.tile_pool(name="normp", bufs=2))
            ostp = ctx.enter_context(tc.tile_pool(name="ostp", bufs=3))

            # ---- persistent SBUF tensors + input DMA ----
            # weights first (small, needed by the very first matmul), then the
            # first 512 columns of every xT chunk (enough for the q-tile-0
            # projections), then the bulk of xT.
            wq_sb = sb.tile([128, 1024], bf16, name="wq_sb", tag="wq_sb")
            wk_sb = sb.tile([128, 1024], bf16, name="wk_sb", tag="wk_sb")
            wv_sb = sb.tile([128, 1024], bf16, name="wv_sb", tag="wv_sb")
            wo_sb = sb.tile([128, 1024], bf16, name="wo_sb", tag="wo_sb")
            nc.sync.dma_start(wk_sb[:], wk_d[:])
            nc.sync.dma_start(wq_sb[:], wq_d[:])
            nc.sync.dma_start(wv_sb[:], wv_d[:])
            nc.sync.dma_start(wo_sb[:], wo_d[:])

            xts = [
                sb.tile([128, S], bf16, name=f"xt{ec}", tag=f"xt{ec}")
                for ec in range(8)
            ]
            for ec in range(8):
                nc.sync.dma_start(
                    xts[ec][:, 0 : 2 * QT], xT_d[ec * 128 : (ec + 1) * 128, 0 : 2 * QT]
                )
            for ec in range(8):
                nc.sync.dma_start(
                    xts[ec][:, 2 * QT : S], xT_d[ec * 128 : (ec + 1) * 128, 2 * QT : S]
                )

            kts = [sb.tile([128, QT], bf16, name=f"kt{i}", tag=f"kt{i}") for i in range(NQ)]
            qts = [sb.tile([128, QT], bf16, name=f"qt{i}", tag=f"qt{i}") for i in range(NQ)]
            aots = [sb.tile([128, QT], bf16, name=f"ao{i}", tag=f"ao{i}") for i in range(NQ)]
            # V for DoubleRow PV: one fp8 tile per kv-block PAIR, layout
            # [128(s within block), pair-slot(2) x 160]: head A V at d 0-63 +
            # ones col 64; head B V at 80-143 + ones col 144 (pair-slot
            # stride 160 B keeps the DoubleRow 16B-alignment rule).
            v8s = []
            for i in range(16):
                v = sb.tile([128, 320], fp8, name=f"v{i}", tag=f"v{i}")
                vv = v[:].rearrange("p (t d) -> p t d", t=2)
                nc.vector.memset(vv[:, :, 64:65], 1.0)
                nc.vector.memset(vv[:, :, 144:145], 1.0)
                v8s.append(v)
            # bf16 V for q-tile 0 (its rows have little context, so fp8
            # attention noise doesn't average out there -> keep bf16)
            vb16 = []
            for i in range(4):
                v = sb.tile([128, 130], bf16, name=f"vb{i}", tag=f"vb{i}")
                nc.vector.memset(v[:, 64:65], 1.0)
                nc.vector.memset(v[:, 129:130], 1.0)
                vb16.append(v)

            # seed tile for the denominator-reciprocal broadcast
            bcseed = sb.tile([64, QT], f32, name="bcseed", tag="bcseed")
            nc.vector.memset(bcseed[:], 0.0)
            # per-partition bias column for the exp range shift
            ebias = sb.tile([128, 1], f32, name="ebias", tag="ebias")
            nc.vector.memset(ebias[:], EXP_BIAS)

            # ---- filler-unit constructors (projections / o_proj) ----
            def kq_units(dst, w, st):
                cols = slice(st * QT, (st + 1) * QT)
                state = {}

                def mm(ec):
                    def f():
                        if ec == 0:
                            state["t"] = ps.tile([128, QT], f32, name="ps_kq", tag="ps")
                        nc.tensor.matmul(
                            state["t"][:],
                            lhsT=w[:, ec * 128 : (ec + 1) * 128],
                            rhs=xts[ec][:, cols],
                            start=(ec == 0),
                            stop=(ec == 7),
                        )

                    return f

                def cast():
                    nc.vector.tensor_copy(dst[:], state["t"][:])

                return [mm(ec) for ec in range(8)] + [cast]

            def v_units(kb):
                state = {}

                def mm(ec):
                    def f():
                        if ec == 0:
                            state["t"] = ps.tile([128, 128], f32, name="ps_v", tag="ps")
                        nc.tensor.matmul(
                            state["t"][:],
                            lhsT=xts[ec][:, kb * 128 : (kb + 1) * 128],
                            rhs=wv_sb[:, ec * 128 : (ec + 1) * 128],
                            start=(ec == 0),
                            stop=(ec == 7),
                        )

                    return f

                def cast():
                    vv = v8s[kb // 2][:].rearrange("p (t d) -> p t d", t=2)
                    r = kb % 2
                    nc.vector.tensor_copy(vv[:, r, 0:64], state["t"][:, 0:64])
                    nc.vector.tensor_copy(vv[:, r, 80:144], state["t"][:, 64:128])
                    if kb < 4:
                        nc.vector.tensor_copy(vb16[kb][:, 0:64], state["t"][:, 0:64])
                        nc.vector.tensor_copy(vb16[kb][:, 65:129], state["t"][:, 64:128])

                return [mm(ec) for ec in range(8)] + [cast]

            def oproj_units(qj, sbis=range(4)):
                units = []
                for sbi in sbis:
                    for half in range(2):

                        def f(sbi=sbi, half=half):
                            srow = qj * QT + sbi * 128
                            po = ps.tile([128, 512], f32, name="po", tag="ps")
                            nc.tensor.matmul(
                                po[:],
                                lhsT=aots[qj][:, sbi * 128 : (sbi + 1) * 128],
                                rhs=wo_sb[:, half * 512 : (half + 1) * 512],
                                start=True,
                                stop=True,
                            )
                            ost = ostp.tile([128, 512], bf16, name="ost", tag="ost")
                            nc.vector.tensor_copy(ost[:], po[:])
                            nc.sync.dma_start(
                                out_d[srow : srow + 128, half * 512 : (half + 1) * 512],
                                ost[:],
                            )

                        units.append(f)
                return units

            def proj_units(qi2):
                u = []
                u += kq_units(kts[qi2], wk_sb, qi2)
                u += kq_units(qts[qi2], wq_sb, qi2)
                for kb in range(4 * qi2, 4 * qi2 + 4):
                    u += v_units(kb)
                return u

            def norm_emit(pvA, pvB, qj, c0, c1):
                # aot[:, c0:c1] = pv[0:64, c0:c1] / pv[64, c0:c1] via
                # approx-reciprocal + quadrant broadcast (stream_shuffle)
                for pv, r0 in ((pvA, 0), (pvB, 64)):
                    den_sb = normp.tile([1, QT], f32, name="den_sb", tag="den")
                    nc.vector.tensor_copy(den_sb[:, c0:c1], pv[64:65, c0:c1])
                    nc.vector.reciprocal_approx_fast(
                        bcseed[0:1, c0:c1], den_sb[:, c0:c1]
                    )
                    nc.vector.tensor_copy(bcseed[32:33, c0:c1], bcseed[0:1, c0:c1])
                    bcast = normp.tile([64, QT], f32, name="bcast", tag="bcast")
                    nc.vector.stream_shuffle(
                        bcast[:, c0:c1], bcseed[:, c0:c1], [0] * 32
                    )
                    nc.vector.tensor_mul(
                        aots[qj][r0 : r0 + 64, c0:c1], pv[0:64, c0:c1],
                        bcast[:, c0:c1],
                    )

            # ---- prologue: projections for q-tile 0 (dense PE warmup) ----
            for f in proj_units(0):
                f()

            # ---- main loop over q-tiles ----
            for qi in range(NQ):
                fillers = deque()
                if qi + 1 < NQ:
                    fillers.extend(kq_units(kts[qi + 1], wk_sb, qi + 1))
                    fillers.extend(kq_units(qts[qi + 1], wq_sb, qi + 1))
                    for kb in range(4 * qi + 4, 4 * qi + 8):
                        fillers.extend(v_units(kb))
                if qi >= 1:
                    fillers.extend(oproj_units(qi - 1))

                n_kb = 4 * (qi + 1)
                pvA = pvp.tile([65, QT], f32, name="pvA", tag="pv")
                pvB = pvp.tile([65, QT], f32, name="pvB", tag="pv")
                if qi == 0:
                    # bf16 path for the first q-tile (rows 0-511): every
                    # block is diagonal; per-block M=65 PV, no DoubleRow.
                    for kb in range(4):
                        kvs = slice(kb * KB, (kb + 1) * KB)
                        off = kb
                        qlo = off * KB
                        nq = QT - qlo
                        lg = lgp.tile([128, 2 * QT], f32, name="lg", tag="lg")
                        nc.tensor.matmul(
                            lg[:, qlo:QT], lhsT=kts[0][0:64, kvs],
                            rhs=qts[0][0:64, qlo:QT], start=True, stop=True,
                        )
                        nc.tensor.matmul(
                            lg[:, QT + qlo : 2 * QT], lhsT=kts[0][64:128, kvs],
                            rhs=qts[0][64:128, qlo:QT], start=True, stop=True,
                        )
                        exb = expp.tile([128, 2 * QT], bf16, name="exb", tag="exp")
                        lg_v = lg[:].rearrange("p (h q) -> p h q", h=2)[:, :, qlo:QT]
                        exb_v = exb[:].rearrange("p (h q) -> p h q", h=2)[:, :, qlo:QT]
                        nc.scalar.activation(exb_v, lg_v, Exp, scale=SCALE,
                                             bias=ebias[:])
                        if fillers:
                            n_pop = math.ceil(len(fillers) / (4 - kb))
                            for _ in range(n_pop):
                                fillers.popleft()()
                        nc.gpsimd.affine_select(
                            out=exb_v, in_=exb_v,
                            compare_op=mybir.AluOpType.is_ge,
                            fill=0.0, base=0,
                            pattern=[[0, 2], [1, nq]],
                            channel_multiplier=-1,
                        )
                        nc.tensor.matmul(
                            pvA[:, qlo:QT], lhsT=vb16[kb][:, 0:65],
                            rhs=exb[:, qlo:QT],
                            start=(kb == 0), stop=(kb == 3),
                            skip_group_check=True,
                        )
                        nc.tensor.matmul(
                            pvB[:, qlo:QT], lhsT=vb16[kb][:, 65:130],
                            rhs=exb[:, QT + qlo : 2 * QT],
                            start=(kb == 0), stop=(kb == 3),
                            skip_group_check=True,
                        )
                    n_kb = 0  # skip the fp8 loop below

                ex8 = None
                for kb in range(n_kb):
                    # logits for both heads of kv-block kb: head A -> cols
                    # 0:512 (PSUM bank 0), head B -> cols 512:1024 (bank 1).
                    # Row-group packing (rows 0-63 / 64-127) runs the two
                    # matmuls concurrently in the PE array.
                    # Diagonal-band pairs: columns q < qlo_p = (pair off)*128
                    # are entirely masked for both members -> skip them.
                    kvs = slice((kb % 4) * KB, (kb % 4 + 1) * KB)
                    ktile = kts[kb // 4]
                    r = kb % 2
                    off = kb - 4 * qi
                    qlo = max(off - r, 0) * KB  # pair-aligned trim
                    nq = QT - qlo
                    lg = lgp.tile([128, 2 * QT], f32, name="lg", tag="lg")
                    nc.tensor.matmul(
                        lg[:, qlo:QT], lhsT=ktile[0:64, kvs],
                        rhs=qts[qi][0:64, qlo:QT],
                        start=True, stop=True,
                    )
                    nc.tensor.matmul(
                        lg[:, QT + qlo : 2 * QT], lhsT=ktile[64:128, kvs],
                        rhs=qts[qi][64:128, qlo:QT],
                        start=True, stop=True,
                    )
                    if r == 0:
                        # exp tile for this kv pair: [128, (head, slot, q)]
                        ex8 = expp.tile([128, 4 * QT], fp8, name="ex8", tag="exp")
                    exv = ex8[:].rearrange("p (h t q) -> p h t q", h=2, t=2)
                    lg_v = lg[:].rearrange("p (h q) -> p h q", h=2)[:, :, qlo:QT]
                    ex_v = exv[:, :, r, qlo:QT]
                    nc.scalar.activation(ex_v, lg_v, Exp, scale=SCALE, bias=ebias[:])

                    # PE filler work while ACT computes exp
                    if fillers:
                        n_pop = math.ceil(len(fillers) / (n_kb - kb))
                        for _ in range(n_pop):
                            fillers.popleft()()

                    if off >= 0:
                        # causal mask on GPSIMD: keep where q - kv - off*128
                        # >= 0 else 0; with q = qlo + j this is
                        # j + qlo - off*128 - kv >= 0.
                        nc.gpsimd.affine_select(
                            out=ex_v,
                            in_=ex_v,
                            compare_op=mybir.AluOpType.is_ge,
                            fill=0.0,
                            base=qlo - off * KB,
                            pattern=[[0, 2], [1, nq]],
                            channel_multiplier=-1,
                        )
                    if r == 1:
                        # DoubleRow PV over the kv pair (contraction 256):
                        # lhsT [128, 2, 65], rhs [128, 2, nq] -> out [65, nq]
                        kp = kb // 2
                        vv = v8s[kp][:].rearrange("p (t d) -> p t d", t=2)
                        nc.tensor.matmul(
                            pvA[:, qlo:QT], lhsT=vv[:, :, 0:65],
                            rhs=exv[:, 0, :, qlo:QT],
                            start=(kp == 0), stop=(kb == n_kb - 1),
                            perf_mode=DoubleRow,
                            skip_group_check=True,
                        )
                        nc.tensor.matmul(
                            pvB[:, qlo:QT], lhsT=vv[:, :, 80:145],
                            rhs=exv[:, 1, :, qlo:QT],
                            start=(kp == 0), stop=(kb == n_kb - 1),
                            perf_mode=DoubleRow,
                            skip_group_check=True,
                        )
                        if qi == NQ - 1 and kb == n_kb - 3:
                            # final tile: the last pair only touches columns
                            # 256+, so columns 0-255 of pv are final now —
                            # normalize them and start their o_proj while
                            # the last pair's attention still runs.
                            norm_emit(pvA, pvB, qi, 0, QT // 2)
                            for f in oproj_units(qi, sbis=(0, 1)):
                                f()
                while fillers:
                    fillers.popleft()()
                # normalize: aot = pv[0:64] / pv[64]
                if qi == NQ - 1:
                    norm_emit(pvA, pvB, qi, QT // 2, QT)
                else:
                    norm_emit(pvA, pvB, qi, 0, QT)

            # epilogue: o_proj of the final tile's second half
            for f in oproj_units(NQ - 1, sbis=(2, 3)):
                f()

    nc.compile()
    return nc


def _host_inputs(x, Wq, Wk, Wv, Wo):
    x2 = np.asarray(x, dtype=np.float32).reshape(S, E)
    xT = np.ascontiguousarray(x2.T).astype(BF16)

    in_maps = []
    for c in range(NCORES):
        r = slice(128 * c, 128 * (c + 1))

        def pack(wT):  # [1024(e), 128(d)] -> [128(p), ec*128+d]
            return np.ascontiguousarray(
                wT.reshape(8, 128, 128).transpose(1, 0, 2).reshape(128, 1024)
            ).astype(BF16)

        wq_c = pack(np.asarray(Wq, np.float32)[r, :].T)
        wk_c = pack(np.asarray(Wk, np.float32)[r, :].T)
        wv_c = pack(np.asarray(Wv, np.float32)[r, :].T)
        wo_c = np.ascontiguousarray(np.asarray(Wo, np.float32)[:, r].T).astype(BF16)
        in_maps.append(
            {
                "xT": xT,
                "wq": wq_c,
                "wk": wk_c,
                "wv": wv_c,
                "wo": wo_c,
            }
        )
    return in_maps


def _get_nc():
    if "nc" not in _CACHE:
        _CACHE["nc"] = _build_nc()
    return _CACHE["nc"]


def run(x, Wq, Wk, Wv, Wo, trace=False, trace_kwargs=None):
    """Build+run the SPMD kernel; returns (full_output [S,E] f32, BassKernelResults)."""
    from concourse.bass_utils import run_bass_kernel_spmd

    nc = _get_nc()
    in_maps = _host_inputs(x, Wq, Wk, Wv, Wo)
    res = run_bass_kernel_spmd(
        nc,
        in_maps,
        list(range(NCORES)),
        trace=trace,
        **(trace_kwargs or {}),
    )
    out = np.zeros((S, E), dtype=np.float32)
    for c in range(NCORES):
        out += res.results[c]["out"].astype(np.float32)
    return out, res


def kernel(x, Wq, Wk, Wv, Wo):
    out, _ = run(x, Wq, Wk, Wv, Wo)
    return out.reshape(1, S, E).astype(np.float32)


# revision 39
# speedup vs baseline: 1.0747x; 1.0118x over previous
"""Causal self-attention Trainium2 kernel (B=1, S=4096, E=1024, H=16, D=64).

Sharding: tensor-parallel over heads — 2 heads per core (8 cores).
Each core computes Q/K/V for its 2 heads, causal attention, and a partial
o_proj over its 128 output-feature slice; the host sums the 8 partials.

Device-side structure (per core):
  * x arrives pre-transposed as xT [E, S] bf16 (host does the transpose),
    so every matmul contracts over the partition axis with contiguous DMAs.
  * Q/K kept transposed in SBUF (qts/kts: [128(d of 2 heads), 512] tiles);
    V in normal layout ([128(s), 64+1] tiles, ones column appended so the
    PV matmul also accumulates the softmax denominator in PSUM row 64).
  * Logits computed transposed, lg[kv, q] = K @ Q.T, both heads packed
    into PE row-groups (tile_position rows 0/64) writing separate banks.
  * exp on ScalarE over [128, 1024] PSUM->SBUF (scale folded in); no
    max-subtraction (logits ~ N(0,1)). Causal masking multiplies the 4
    diagonal-band blocks per q-tile by 0/1 masks.
  * Normalize via reciprocal_approx_fast + stream_shuffle broadcast.
  * The per-q-tile QKV projections and o_proj matmuls are interleaved as
    PE "filler" work between attention pairs so the PE never idles long
    enough for the HAM clock gate to re-throttle it.
"""

import math
import sys
from collections import deque

import numpy as np

for _p in ("/opt/trn_rl_repo", "/opt/trn_rl_repo/concourse"):
    if _p not in sys.path:
        sys.path.insert(0, _p)

import ml_dtypes

BF16 = ml_dtypes.bfloat16

S = 4096
E = 1024
H = 16
D = 64
NCORES = 8
DH = 128  # head dims per core (2 heads x 64)
QT = 512  # query tile (free dim of logits matmuls)
NQ = S // QT  # 8
KB = 128  # kv block (partition dim of logits tiles)
SCALE = 1.0 / math.sqrt(D)

_CACHE = {}


def _build_nc():
    import concourse.tile as tile
    from concourse import bacc, mybir

    dt = mybir.dt
    f32 = dt.float32
    bf16 = dt.bfloat16
    fp8 = dt.float8e4
    Exp = mybir.ActivationFunctionType.Exp
    DoubleRow = mybir.MatmulPerfMode.DoubleRow
    # exp(scale*logit + EXP_BIAS): keeps exp values well inside fp8e4's
    # +-240 range; the uniform e^bias factor cancels in the normalization.
    EXP_BIAS = -3.5

    nc = bacc.Bacc("TRN2", target_bir_lowering=False, debug=False, num_devices=NCORES)

    xT_d = nc.dram_tensor("xT", [E, S], bf16, kind="ExternalInput")
    wq_d = nc.dram_tensor("wq", [128, 1024], bf16, kind="ExternalInput")
    wk_d = nc.dram_tensor("wk", [128, 1024], bf16, kind="ExternalInput")
    wv_d = nc.dram_tensor("wv", [128, 1024], bf16, kind="ExternalInput")
    wo_d = nc.dram_tensor("wo", [128, 1024], bf16, kind="ExternalInput")
    out_d = nc.dram_tensor("out", [S, E], bf16, kind="ExternalOutput")

    with tile.TileContext(nc) as tc:
        from contextlib import ExitStack

        with ExitStack() as ctx:
            sb = ctx.enter_context(tc.tile_pool(name="sb", bufs=1))
            lgp = ctx.enter_context(tc.tile_pool(name="lgp", bufs=2, space="PSUM"))
            ps = ctx.enter_context(tc.tile_pool(name="ps", bufs=2, space="PSUM"))
            pvp = ctx.enter_context(tc.tile_pool(name="pvp", bufs=2, space="PSUM"))
            expp = ctx.enter_context(tc.tile_pool(name="expp", bufs=6))
            normp = ctx.enter_context(tc.tile_pool(name="normp", bufs=3))
            ostp = ctx.enter_context(tc.tile_pool(name="ostp", bufs=4))

            # ---- persistent SBUF tensors + input DMA ----
            # weights first (small, needed by the very first matmul), then the
            # first 512 columns of every xT chunk (enough for the q-tile-0
            # projections), then the bulk of xT.
            wq_sb = sb.tile([128, 1024], bf16, name="wq_sb", tag="wq_sb")
            wk_sb = sb.tile([128, 1024], bf16, name="wk_sb", tag="wk_sb")
            wv_sb = sb.tile([128, 1024], bf16, name="wv_sb", tag="wv_sb")
            wo_sb = sb.tile([128, 1024], bf16, name="wo_sb", tag="wo_sb")
            nc.sync.dma_start(wk_sb[:], wk_d[:])
            nc.sync.dma_start(wq_sb[:], wq_d[:])
            nc.sync.dma_start(wv_sb[:], wv_d[:])
            nc.sync.dma_start(wo_sb[:], wo_d[:])

            xts = [
                sb.tile([128, S], bf16, name=f"xt{ec}", tag=f"xt{ec}")
                for ec in range(8)
            ]
            for ec in range(8):
                nc.sync.dma_start(
                    xts[ec][:, 0 : 2 * QT], xT_d[ec * 128 : (ec + 1) * 128, 0 : 2 * QT]
                )
            for ec in range(8):
                nc.sync.dma_start(
                    xts[ec][:, 2 * QT : S], xT_d[ec * 128 : (ec + 1) * 128, 2 * QT : S]
                )

            kts = [sb.tile([128, QT], bf16, name=f"kt{i}", tag=f"kt{i}") for i in range(NQ)]
            qts = [sb.tile([128, QT], bf16, name=f"qt{i}", tag=f"qt{i}") for i in range(NQ)]
            aots = [sb.tile([128, QT], bf16, name=f"ao{i}", tag=f"ao{i}") for i in range(NQ)]
            # V for DoubleRow PV: one fp8 tile per kv-block PAIR, layout
            # [128(s within block), pair-slot(2) x 160]: head A V at d 0-63 +
            # ones col 64; head B V at 80-143 + ones col 144 (pair-slot
            # stride 160 B keeps the DoubleRow 16B-alignment rule).
            v8s = []
            for i in range(16):
                v = sb.tile([128, 320], fp8, name=f"v{i}", tag=f"v{i}")
                vv = v[:].rearrange("p (t d) -> p t d", t=2)
                nc.vector.memset(vv[:, :, 64:65], 1.0)
                nc.vector.memset(vv[:, :, 144:145], 1.0)
                v8s.append(v)
            # bf16 V for q-tile 0 (its rows have little context, so fp8
            # attention noise doesn't average out there -> keep bf16)
            vb16 = []
            for i in range(4):
                v = sb.tile([128, 130], bf16, name=f"vb{i}", tag=f"vb{i}")
                nc.vector.memset(v[:, 64:65], 1.0)
                nc.vector.memset(v[:, 129:130], 1.0)
                vb16.append(v)

            # seed tile for the denominator-reciprocal broadcast
            bcseed = sb.tile([64, QT], f32, name="bcseed", tag="bcseed")
            nc.vector.memset(bcseed[:], 0.0)
            # per-partition bias column for the exp range shift
            ebias = sb.tile([128, 1], f32, name="ebias", tag="ebias")
            nc.vector.memset(ebias[:], EXP_BIAS)

            # ---- filler-unit constructors (projections / o_proj) ----
            def kq_units(dst, w, st):
                cols = slice(st * QT, (st + 1) * QT)
                state = {}

                def mm(ec):
                    def f():
                        if ec == 0:
                            state["t"] = ps.tile([128, QT], f32, name="ps_kq", tag="ps")
                        nc.tensor.matmul(
                            state["t"][:],
                            lhsT=w[:, ec * 128 : (ec + 1) * 128],
                            rhs=xts[ec][:, cols],
                            start=(ec == 0),
                            stop=(ec == 7),
                        )

                    return f

                def cast():
                    nc.vector.tensor_copy(dst[:], state["t"][:])

                return [mm(ec) for ec in range(8)] + [cast]

            def v_units(kb):
                state = {}

                def mm(ec):
                    def f():
                        if ec == 0:
                            state["t"] = ps.tile([128, 128], f32, name="ps_v", tag="ps")
                        nc.tensor.matmul(
                            state["t"][:],
                            lhsT=xts[ec][:, kb * 128 : (kb + 1) * 128],
                            rhs=wv_sb[:, ec * 128 : (ec + 1) * 128],
                            start=(ec == 0),
                            stop=(ec == 7),
                        )

                    return f

                def cast():
                    vv = v8s[kb // 2][:].rearrange("p (t d) -> p t d", t=2)
                    r = kb % 2
                    nc.vector.tensor_copy(vv[:, r, 0:64], state["t"][:, 0:64])
                    nc.vector.tensor_copy(vv[:, r, 80:144], state["t"][:, 64:128])
                    if kb < 4:
                        nc.vector.tensor_copy(vb16[kb][:, 0:64], state["t"][:, 0:64])
                        nc.vector.tensor_copy(vb16[kb][:, 65:129], state["t"][:, 64:128])

                return [mm(ec) for ec in range(8)] + [cast]

            def oproj_units(qj, sbis=range(4)):
                units = []
                for sbi in sbis:
                    for half in range(2):

                        def f(sbi=sbi, half=half):
                            srow = qj * QT + sbi * 128
                            po = ps.tile([128, 512], f32, name="po", tag="ps")
                            nc.tensor.matmul(
                                po[:],
                                lhsT=aots[qj][:, sbi * 128 : (sbi + 1) * 128],
                                rhs=wo_sb[:, half * 512 : (half + 1) * 512],
                                start=True,
                                stop=True,
                            )
                            ost = ostp.tile([128, 512], bf16, name="ost", tag="ost")
                            nc.vector.tensor_copy(ost[:], po[:])
                            nc.sync.dma_start(
                                out_d[srow : srow + 128, half * 512 : (half + 1) * 512],
                                ost[:],
                            )

                        units.append(f)
                return units

            def proj_units(qi2):
                u = []
                u += kq_units(kts[qi2], wk_sb, qi2)
                u += kq_units(qts[qi2], wq_sb, qi2)
                for kb in range(4 * qi2, 4 * qi2 + 4):
                    u += v_units(kb)
                return u

            def norm_emit(pvA, pvB, qj, c0, c1):
                # aot[:, c0:c1] = pv[0:64, c0:c1] / pv[64, c0:c1] via
                # approx-reciprocal + quadrant broadcast (stream_shuffle)
                for pv, r0 in ((pvA, 0), (pvB, 64)):
                    den_sb = normp.tile([1, QT], f32, name="den_sb", tag="den")
                    nc.vector.tensor_copy(den_sb[:, c0:c1], pv[64:65, c0:c1])
                    nc.vector.reciprocal_approx_fast(
                        bcseed[0:1, c0:c1], den_sb[:, c0:c1]
                    )
                    nc.vector.tensor_copy(bcseed[32:33, c0:c1], bcseed[0:1, c0:c1])
                    bcast = normp.tile([64, QT], f32, name="bcast", tag="bcast")
                    nc.vector.stream_shuffle(
                        bcast[:, c0:c1], bcseed[:, c0:c1], [0] * 32
                    )
                    nc.vector.tensor_mul(
                        aots[qj][r0 : r0 + 64, c0:c1], pv[0:64, c0:c1],
                        bcast[:, c0:c1],
                    )

            # ---- prologue: projections for q-tile 0 (dense PE warmup) ----
            for f in proj_units(0):
                f()

            # ---- main loop over q-tiles ----
            for qi in range(NQ):
                fillers = deque()
                if qi + 1 < NQ:
                    fillers.extend(kq_units(kts[qi + 1], wk_sb, qi + 1))
                    fillers.extend(kq_units(qts[qi + 1], wq_sb, qi + 1))
                    for kb in range(4 * qi + 4, 4 * qi + 8):
                        fillers.extend(v_units(kb))
                if qi >= 1:
                    fillers.extend(oproj_units(qi - 1))

                n_kb = 4 * (qi + 1)
                pvA = pvp.tile([65, QT], f32, name="pvA", tag="pv")
                pvB = pvp.tile([65, QT], f32, name="pvB", tag="pv")
                if qi == 0:
                    # bf16 path for the first q-tile (rows 0-511): every
                    # block is diagonal; per-block M=65 PV, no DoubleRow.
                    for kb in range(4):
                        kvs = slice(kb * KB, (kb + 1) * KB)
                        off = kb
                        qlo = off * KB
                        nq = QT - qlo
                        lg = lgp.tile([128, 2 * QT], f32, name="lg", tag="lg")
                        nc.tensor.matmul(
                            lg[:, qlo:QT], lhsT=kts[0][0:64, kvs],
                            rhs=qts[0][0:64, qlo:QT], start=True, stop=True,
                        )
                        nc.tensor.matmul(
                            lg[:, QT + qlo : 2 * QT], lhsT=kts[0][64:128, kvs],
                            rhs=qts[0][64:128, qlo:QT], start=True, stop=True,
                        )
                        exb = expp.tile([128, 2 * QT], bf16, name="exb", tag="exp")
                        lg_v = lg[:].rearrange("p (h q) -> p h q", h=2)[:, :, qlo:QT]
                        exb_v = exb[:].rearrange("p (h q) -> p h q", h=2)[:, :, qlo:QT]
                        nc.scalar.activation(exb_v, lg_v, Exp, scale=SCALE,
                                             bias=ebias[:])
                        if fillers:
                            n_pop = math.ceil(len(fillers) / (4 - kb))
                            for _ in range(n_pop):
                                fillers.popleft()()
                        nc.gpsimd.affine_select(
                            out=exb_v, in_=exb_v,
                            compare_op=mybir.AluOpType.is_ge,
                            fill=0.0, base=0,
                            pattern=[[0, 2], [1, nq]],
                            channel_multiplier=-1,
                        )
                        nc.tensor.matmul(
                            pvA[:, qlo:QT], lhsT=vb16[kb][:, 0:65],
                            rhs=exb[:, qlo:QT],
                            start=(kb == 0), stop=(kb == 3),
                            skip_group_check=True,
                        )
                        nc.tensor.matmul(
                            pvB[:, qlo:QT], lhsT=vb16[kb][:, 65:130],
                            rhs=exb[:, QT + qlo : 2 * QT],
                            start=(kb == 0), stop=(kb == 3),
                            skip_group_check=True,
                        )
                    n_kb = 0  # skip the fp8 loop below

                ex8 = None
                for kb in range(n_kb):
                    # logits for both heads of kv-block kb: head A -> cols
                    # 0:512 (PSUM bank 0), head B -> cols 512:1024 (bank 1).
                    # Row-group packing (rows 0-63 / 64-127) runs the two
                    # matmuls concurrently in the PE array.
                    # Diagonal-band pairs: columns q < qlo_p = (pair off)*128
                    # are entirely masked for both members -> skip them.
                    kvs = slice((kb % 4) * KB, (kb % 4 + 1) * KB)
                    ktile = kts[kb // 4]
                    r = kb % 2
                    off = kb - 4 * qi
                    qlo = max(off - r, 0) * KB  # pair-aligned trim
                    nq = QT - qlo
                    lg = lgp.tile([128, 2 * QT], f32, name="lg", tag="lg")
                    nc.tensor.matmul(
                        lg[:, qlo:QT], lhsT=ktile[0:64, kvs],
                        rhs=qts[qi][0:64, qlo:QT],
                        start=True, stop=True,
                    )
                    nc.tensor.matmul(
                        lg[:, QT + qlo : 2 * QT], lhsT=ktile[64:128, kvs],
                        rhs=qts[qi][64:128, qlo:QT],
                        start=True, stop=True,
                    )
                    if r == 0:
                        # exp tile for this kv pair: [128, (head, slot, q)]
                        ex8 = expp.tile([128, 4 * QT], fp8, name="ex8", tag="exp")
                    exv = ex8[:].rearrange("p (h t q) -> p h t q", h=2, t=2)
                    lg_v = lg[:].rearrange("p (h q) -> p h q", h=2)[:, :, qlo:QT]
                    ex_v = exv[:, :, r, qlo:QT]
                    nc.scalar.activation(ex_v, lg_v, Exp, scale=SCALE, bias=ebias[:])

                    # PE filler work while ACT computes exp
                    if fillers:
                        n_pop = math.ceil(len(fillers) / (n_kb - kb))
                        for _ in range(n_pop):
                            fillers.popleft()()

                    if off >= 0:
                        # causal mask on GPSIMD: keep where q - kv - off*128
                        # >= 0 else 0; with q = qlo + j this is
                        # j + qlo - off*128 - kv >= 0.
                        nc.gpsimd.affine_select(
                            out=ex_v,
                            in_=ex_v,
                            compare_op=mybir.AluOpType.is_ge,
                            fill=0.0,
                            base=qlo - off * KB,
                            pattern=[[0, 2], [1, nq]],
                            channel_multiplier=-1,
                        )
                    if r == 1:
                        # DoubleRow PV over the kv pair (contraction 256):
                        # lhsT [128, 2, 65], rhs [128, 2, nq] -> out [65, nq]
                        kp = kb // 2
                        vv = v8s[kp][:].rearrange("p (t d) -> p t d", t=2)
                        nc.tensor.matmul(
                            pvA[:, qlo:QT], lhsT=vv[:, :, 0:65],
                            rhs=exv[:, 0, :, qlo:QT],
                            start=(kp == 0), stop=(kb == n_kb - 1),
                            perf_mode=DoubleRow,
                            skip_group_check=True,
                        )
                        nc.tensor.matmul(
                            pvB[:, qlo:QT], lhsT=vv[:, :, 80:145],
                            rhs=exv[:, 1, :, qlo:QT],
                            start=(kp == 0), stop=(kb == n_kb - 1),
                            perf_mode=DoubleRow,
                            skip_group_check=True,
                        )
                        if qi == NQ - 1 and kb == n_kb - 3:
                            # final tile: the last pair only touches columns
                            # 256+, so columns 0-255 of pv are final now —
                            # normalize them and start their o_proj while
                            # the last pair's attention still runs.
                            norm_emit(pvA, pvB, qi, 0, QT // 2)
                            for f in oproj_units(qi, sbis=(0, 1)):
                                f()
                while fillers:
                    fillers.popleft()()
                # normalize: aot = pv[0:64] / pv[64]
                if qi == NQ - 1:
                    norm_emit(pvA, pvB, qi, QT // 2, QT)
                else:
                    norm_emit(pvA, pvB, qi, 0, QT)

            # epilogue: o_proj of the final tile's second half
            for f in oproj_units(NQ - 1, sbis=(2, 3)):
                f()

    nc.compile()
    return nc


def _host_inputs(x, Wq, Wk, Wv, Wo):
    x2 = np.asarray(x, dtype=np.float32).reshape(S, E)
    xT = np.ascontiguousarray(x2.T).astype(BF16)

    in_maps = []
    for c in range(NCORES):
        r = slice(128 * c, 128 * (c + 1))

        def pack(wT):  # [1024(e), 128(d)] -> [128(p), ec*128+d]
            return np.ascontiguousarray(
                wT.reshape(8, 128, 128).transpose(1, 0, 2).reshape(128, 1024)
            ).astype(BF16)

        wq_c = pack(np.asarray(Wq, np.float32)[r, :].T)
        wk_c = pack(np.asarray(Wk, np.float32)[r, :].T)
        wv_c = pack(np.asarray(Wv, np.float32)[r, :].T)
        wo_c = np.ascontiguousarray(np.asarray(Wo, np.float32)[:, r].T).astype(BF16)
        in_maps.append(
            {
                "xT": xT,
                "wq": wq_c,
                "wk": wk_c,
                "wv": wv_c,
                "wo": wo_c,
            }
        )
    return in_maps


def _get_nc():
    if "nc" not in _CACHE:
        _CACHE["nc"] = _build_nc()
    return _CACHE["nc"]


def run(x, Wq, Wk, Wv, Wo, trace=False, trace_kwargs=None):
    """Build+run the SPMD kernel; returns (full_output [S,E] f32, BassKernelResults)."""
    from concourse.bass_utils import run_bass_kernel_spmd

    nc = _get_nc()
    in_maps = _host_inputs(x, Wq, Wk, Wv, Wo)
    res = run_bass_kernel_spmd(
        nc,
        in_maps,
        list(range(NCORES)),
        trace=trace,
        **(trace_kwargs or {}),
    )
    out = np.zeros((S, E), dtype=np.float32)
    for c in range(NCORES):
        out += res.results[c]["out"].astype(np.float32)
    return out, res


def kernel(x, Wq, Wk, Wv, Wo):
    out, _ = run(x, Wq, Wk, Wv, Wo)
    return out.reshape(1, S, E).astype(np.float32)
